# revision 1
# baseline (speedup 1.0000x reference)
"""CCNet unit (conv3x3 -> BN/ReLU -> 2x criss-cross attention -> conv3x3 ->
BN/ReLU) on 8 Trainium2 NeuronCores.

Sharding (SPMD-symmetric program; per-core differences live only in data):
  core = 2*b + half   (b = sample 0..3, half = 0/1)
  - conv1 (Cin=2048): input-channel split across the pair; partial sums
    pair-AllReduce'd (fp16), then BN+ReLU on both cores.
  - CCA x2: computed redundantly by both pair members (cheap vs conv1).
  - conv2 (Cout=512): output-channel split via sharded weights.

All matmuls fp16 (full PE rate), PSUM accumulates fp32.

Attention layouts (parity-packed so matmul operand base partitions match):
  expT  [128, 2048] : exp(eH^T) at [64*(w%2)+i, (w//2)*64 + h], diag-masked
  expTW [128, 2048] : exp(eW^T) at [64*(h%2)+j, (h//2)*64 + w]
  v_tc  [128, t, C] : v at [64*(w%2)+h, w//2, c]   (column-major pixels)
  v_tr  [128, t, C] : v at [64*(h%2)+w, h//2, c]   (row-major pixels)
Softmax has no max-subtraction (logits bounded ~ +-5); normalization
(gamma/s) is folded into expT/expTW before aggregation.
"""

import sys

sys.path.insert(0, "/opt/trn_rl_repo")

import numpy as np
import ml_dtypes

import concourse.bacc as bacc
import concourse.mybir as mybir
import concourse.tile as tile
from concourse.bass_utils import run_bass_kernel_spmd

dt = mybir.dt
AF = mybir.ActivationFunctionType
ALU = mybir.AluOpType

B, CIN, C, CR, H, W = 4, 2048, 512, 64, 64, 64
HW = H * W
WP = W + 2
PADPX = (H + 2) * WP
INT0 = WP + 1
KT1 = CIN // 2 // 128   # 8
CT = C // 128           # 4
N_CORES = 8
PAIRS = [[0, 1], [2, 3], [4, 5], [6, 7]]

DEBUG_STAGE = None
CCA_STOP = None  # 'qk'|'exp'|'srows'|'scale'|'aggH'
_COMPILED = {}


def _pad_ap(t, ct):
    """[128, 64, 64] interior view of padded feats tile channel-tile ct."""
    return t[:, ct, INT0:INT0 + WP * H].rearrange("p (h w) -> p h w", w=WP)[:, :, 0:W]


def _chunk_rhs(t, ct, h0, off):
    """[128, 8, 64] rhs AP: 8 output rows from h0, reading offset `off`."""
    r0, c0 = h0 + off // WP, off % WP
    v = t[:, ct, :].rearrange("p (h w) -> p h w", w=WP)
    return v[:, r0:r0 + 8, c0:c0 + 64]


def build_kernel(debug_stage=None, cca_stop=None):
    nc = bacc.Bacc("TRN2", target_bir_lowering=False, debug=False,
                   num_devices=N_CORES)

    x_in = nc.dram_tensor("x", [KT1, 128, HW], dt.float16, kind="ExternalInput").ap()
    w1_in = nc.dram_tensor("w1", [KT1, 9, 128, C], dt.float16, kind="ExternalInput").ap()
    bn1_in = nc.dram_tensor("bn1", [128, CT, 2], dt.float32, kind="ExternalInput").ap()
    qkw_in = nc.dram_tensor("qkw", [CT, 128, 128], dt.float16, kind="ExternalInput").ap()
    qb_in = nc.dram_tensor("qb", [64, 1], dt.float32, kind="ExternalInput").ap()
    vw_in = nc.dram_tensor("vw", [CT, 128, C], dt.float16, kind="ExternalInput").ap()
    gvb_in = nc.dram_tensor("gvb", [128, CT], dt.float32, kind="ExternalInput").ap()
    og_in = nc.dram_tensor("og", [64, 1], dt.bfloat16, kind="ExternalInput").ap()
    mask_in = nc.dram_tensor("mask", [64, 64], dt.bfloat16, kind="ExternalInput").ap()
    w2_in = nc.dram_tensor("w2", [CT, 9, 128, 256], dt.float16, kind="ExternalInput").ap()
    bn2_in = nc.dram_tensor("bn2", [128, 2, 2], dt.float32, kind="ExternalInput").ap()

    if debug_stage is None:
        out_t = nc.dram_tensor("out", [2, 128, HW], dt.float32, kind="ExternalOutput").ap()
    else:
        out_t = nc.dram_tensor("out", [CT, 128, HW], dt.float32, kind="ExternalOutput").ap()

    with tile.TileContext(nc) as tc:
        _emit(nc, tc, debug_stage, cca_stop, x_in, w1_in, bn1_in, qkw_in, qb_in, vw_in,
              gvb_in, og_in, mask_in, w2_in, bn2_in, out_t)
    nc.compile()
    return nc


def _emit(nc, tc, debug_stage, cca_stop, x_in, w1_in, bn1_in, qkw_in, qb_in, vw_in,
          gvb_in, og_in, mask_in, w2_in, bn2_in, out_t):
    from contextlib import ExitStack

    ctx = ExitStack()
    with ctx:
        pool_feats = ctx.enter_context(tc.tile_pool(name="feats", bufs=1))
        featsA = pool_feats.tile([128, CT, PADPX], dt.float16)

        pool_const = ctx.enter_context(tc.tile_pool(name="const", bufs=1))
        bn1 = pool_const.tile([128, CT, 2], dt.float32)
        qkw = pool_const.tile([128, CT, 128], dt.float16)
        qb = pool_const.tile([64, 1], dt.float32)
        vw = pool_const.tile([128, CT, C], dt.float16)
        gvb = pool_const.tile([128, CT], dt.float32)
        og = pool_const.tile([64, 1], dt.bfloat16)
        mask = pool_const.tile([64, 64], dt.bfloat16)
        nc.sync.dma_start(bn1[:], bn1_in[:])
        nc.sync.dma_start(qkw[:], qkw_in[:].rearrange("k p c -> p k c"))
        nc.sync.dma_start(qb[:], qb_in[:])
        nc.sync.dma_start(vw[:], vw_in[:].rearrange("k p c -> p k c"))
        nc.sync.dma_start(gvb[:], gvb_in[:])
        nc.sync.dma_start(og[:], og_in[:])
        nc.sync.dma_start(mask[:], mask_in[:])

        # ---------------- conv1 ----------------
        with (
            tc.tile_pool(name="c1", bufs=1) as c1,
            tc.tile_pool(name="c1ps", bufs=8, space="PSUM") as c1ps,
            tc.tile_pool(name="c1st", bufs=4) as c1st,
            tc.tile_pool(name="c1dram", bufs=1, space="DRAM") as c1dram,
        ):
            nc.vector.memset(featsA[:], 0.0)
            x_pad = c1.tile([128, KT1, PADPX], dt.float16)
            w1 = c1.tile([128, KT1, 9, C], dt.float16)
            nc.vector.memset(x_pad[:], 0.0)
            for kt in range(KT1):
                nc.sync.dma_start(w1[:, kt, :, :],
                                  w1_in[kt].rearrange("t p c -> p t c"))
                nc.sync.dma_start(_pad_ap(x_pad, kt),
                                  x_in[kt].rearrange("p (h w) -> p h w", w=W))

            partial = c1dram.tile([CT, 128, HW], dt.float16)
            reduced = c1dram.tile([CT, 128, HW], dt.float16)

            for mt in range(CT):
                for h0 in range(0, H, 8):
                    ps = c1ps.tile([128, 512], dt.float32)
                    i = 0
                    for kt in range(KT1):
                        for dy in range(3):
                            for dx in range(3):
                                nc.tensor.matmul(
                                    ps[:],
                                    w1[:, kt, dy * 3 + dx, mt * 128:(mt + 1) * 128],
                                    _chunk_rhs(x_pad, kt, h0, dy * WP + dx),
                                    start=(i == 0), stop=(i == KT1 * 9 - 1))
                                i += 1
                    st = c1st.tile([128, 512], dt.float16)
                    nc.scalar.activation(st[:], ps[:], AF.Copy)
                    nc.sync.dma_start(partial[mt, :, h0 * W:(h0 + 8) * W], st[:])

                nc.gpsimd.collective_compute(
                    "AllReduce", ALU.add, replica_groups=PAIRS,
                    ins=[partial[mt]], outs=[reduced[mt]])
                for h0 in range(0, H, 8):
                    red_sb = c1st.tile([128, 512], dt.float16, tag="redsb")
                    nc.sync.dma_start(red_sb[:], reduced[mt, :, h0 * W:(h0 + 8) * W])
                    nc.scalar.activation(
                        _chunk_rhs(featsA, mt, h0, INT0),
                        red_sb[:].rearrange("p (h w) -> p h w", w=W),
                        AF.Relu, bias=bn1[:, mt, 1:2], scale=bn1[:, mt, 0:1])

        if debug_stage == "feats1":
            _emit_debug_out(nc, tc, featsA, out_t)
            return

        # ---------------- CCA x2 ----------------
        pool_fb = ctx.enter_context(tc.tile_pool(name="featsB", bufs=1))
        featsB = pool_fb.tile([128, CT, PADPX], dt.float16)
        nc.vector.memset(featsB[:], 0.0)
        _emit_cca(nc, tc, featsA, featsB, qkw, qb, vw, gvb, og, mask,
                  cca_stop=cca_stop, dbg_out=(out_t if cca_stop else None))
        if cca_stop:
            return
        if debug_stage == "cca1":
            _emit_debug_out(nc, tc, featsB, out_t)
            return
        _emit_cca(nc, tc, featsB, featsA, qkw, qb, vw, gvb, og, mask)
        if debug_stage == "cca2":
            _emit_debug_out(nc, tc, featsA, out_t)
            return

        # ---------------- conv2 ----------------
        with (
            tc.tile_pool(name="c2", bufs=1) as c2,
            tc.tile_pool(name="c2ps", bufs=8, space="PSUM") as c2ps,
            tc.tile_pool(name="c2st", bufs=4) as c2st,
        ):
            w2 = c2.tile([128, CT, 9, 256], dt.float16)
            bn2 = c2.tile([128, 2, 2], dt.float32)
            nc.sync.dma_start(bn2[:], bn2_in[:])
            for kt in range(CT):
                nc.sync.dma_start(w2[:, kt, :, :],
                                  w2_in[kt].rearrange("t p c -> p t c"))
            for mt in range(2):
                for h0 in range(0, H, 8):
                    ps = c2ps.tile([128, 512], dt.float32)
                    i = 0
                    for kt in range(CT):
                        for dy in range(3):
                            for dx in range(3):
                                nc.tensor.matmul(
                                    ps[:],
                                    w2[:, kt, dy * 3 + dx, mt * 128:(mt + 1) * 128],
                                    _chunk_rhs(featsA, kt, h0, dy * WP + dx),
                                    start=(i == 0), stop=(i == CT * 9 - 1))
                                i += 1
                    st = c2st.tile([128, 512], dt.float32)
                    nc.scalar.activation(st[:], ps[:], AF.Relu,
                                         bias=bn2[:, mt, 1:2],
                                         scale=bn2[:, mt, 0:1])
                    nc.sync.dma_start(out_t[mt, :, h0 * W:(h0 + 8) * W], st[:])


def _emit_debug_out(nc, tc, feats, out_t):
    with tc.tile_pool(name="dbg", bufs=4) as dbg:
        for ct in range(CT):
            st = dbg.tile([128, HW], dt.float32)
            nc.vector.tensor_copy(
                st[:].rearrange("p (h w) -> p h w", w=W), _pad_ap(feats, ct))
            nc.sync.dma_start(out_t[ct], st[:])


def _emit_cca(nc, tc, fin, fout, qkw, qb, vw, gvb, og, mask,
              cca_stop=None, dbg_out=None):
    """fout = gamma*cca(fin) + fin (interior; fout border must be zero).

    All matmul operands live at partition base 0 (base-64 operands crash the
    PE when mixed with base-0 matmuls). Parity-split tensors are separate
    tiles:
      expT_e/expT_o  [64, 2048] : exp(eH^T)[i, (w//2)*64+h] for even/odd w
      expTW_e/expTW_o[64, 2048] : exp(eW^T)[j, (h//2)*64+w] for even/odd h
    """
    from contextlib import ExitStack

    ctx = ExitStack()
    with ctx:
        sb = ctx.enter_context(tc.tile_pool(name="cca_sb", bufs=1))
        sp = ctx.enter_context(tc.tile_pool(name="cca_small", bufs=2))

        expT_e = sb.tile([64, 2048], dt.bfloat16)
        expT_o = sb.tile([64, 2048], dt.bfloat16)
        expTW_e = sb.tile([64, 2048], dt.bfloat16)
        expTW_o = sb.tile([64, 2048], dt.bfloat16)
        expT = (expT_e, expT_o)
        expTW = (expTW_e, expTW_o)

        with (
            tc.tile_pool(name="cca_qk", bufs=1) as qkpool,
            tc.tile_pool(name="cca_qkps", bufs=4, space="PSUM") as qkps,
            tc.tile_pool(name="cca_eps", bufs=4, space="PSUM") as eps,
        ):
            # ---- q, k projections ----
            q_sb = qkpool.tile([64, HW], dt.float16)
            k_sb = qkpool.tile([64, HW], dt.float16)
            for n in range(8):
                ps = qkps.tile([128, 512], dt.float32)
                for kt in range(CT):
                    nc.tensor.matmul(
                        ps[:], qkw[:, kt, :], _chunk_rhs(fin, kt, n * 8, INT0),
                        start=(kt == 0), stop=(kt == CT - 1))
                nc.scalar.activation(q_sb[:, n * 512:(n + 1) * 512], ps[0:64, :],
                                     AF.Identity, bias=qb[:], scale=1.0)
                # k: convert at partitions 64:128, then DMA-move down to 0:63
                kev = sp.tile([128, 512], dt.float16, tag="kev")
                nc.scalar.activation(kev[64:128, :], ps[64:128, :], AF.Copy)
                nc.sync.dma_start(k_sb[:, n * 512:(n + 1) * 512], kev[64:128, :])

            if cca_stop == "qk":
                with tc.tile_pool(name="dbgq", bufs=2) as dbgq:
                    s1 = dbgq.tile([64, HW], dt.float32)
                    nc.vector.tensor_copy(s1[:], q_sb[:])
                    nc.sync.dma_start(dbg_out[0, 0:64, :], s1[:])
                    s2 = dbgq.tile([64, HW], dt.float32)
                    nc.vector.tensor_copy(s2[:], k_sb[:])
                    nc.sync.dma_start(dbg_out[1, 0:64, :], s2[:])
                return

            # ---- e^T matmuls + exp; groups of 4 same-parity columns/rows ----
            kv = k_sb[:].rearrange("p (h w) -> p w h", w=W)
            qv = q_sb[:].rearrange("p (h w) -> p w h", w=W)
            for par in range(2):
                for g in range(8):
                    ps = eps.tile([64, 256], dt.float32, tag="eps")
                    for m in range(4):
                        w = 8 * g + 2 * m + par
                        nc.tensor.matmul(ps[:, m * 64:(m + 1) * 64],
                                         kv[:, w, :], qv[:, w, :],
                                         start=True, stop=True)
                    e16 = sp.tile([64, 256], dt.bfloat16, tag="e16")
                    nc.scalar.activation(e16[:], ps[:], AF.Exp)
                    nc.vector.tensor_mul(
                        expT[par][:, g * 256:(g + 1) * 256].rearrange(
                            "p (a b) -> p a b", a=4),
                        e16[:].rearrange("p (a b) -> p a b", a=4),
                        mask[:, None, :].broadcast_to((64, 4, 64)))
            for par in range(2):
                for g in range(8):
                    ps = eps.tile([64, 256], dt.float32, tag="eps")
                    for m in range(4):
                        h = 8 * g + 2 * m + par
                        nc.tensor.matmul(ps[:, m * 64:(m + 1) * 64],
                                         k_sb[:, h * W:(h + 1) * W],
                                         q_sb[:, h * W:(h + 1) * W],
                                         start=True, stop=True)
                    nc.scalar.activation(
                        expTW[par][:, g * 256:(g + 1) * 256], ps[:], AF.Exp)

        if cca_stop == "exp":
            with tc.tile_pool(name="dbge", bufs=2) as dbge:
                for i, t in enumerate((expT_e, expT_o, expTW_e, expTW_o)):
                    s1 = dbge.tile([64, 2048], dt.float32)
                    nc.vector.tensor_copy(s1[:], t[:])
                    nc.sync.dma_start(dbg_out[i, 0:64, 0:2048], s1[:])
            return

        # ---- denominators ----
        # rowH* hold sH then become rs_rpx2; rowW* hold sW then rs_cpx2.
        rowHe = sp.tile([1, 2048], dt.float32, tag="rowHe", bufs=1)
        rowHo = sp.tile([1, 2048], dt.float32, tag="rowHo", bufs=1)
        rowWe = sp.tile([1, 2048], dt.float32, tag="rowWe", bufs=1)
        rowWo = sp.tile([1, 2048], dt.float32, tag="rowWo", bufs=1)
        with tc.tile_pool(name="cca_sps", bufs=4, space="PSUM") as sps:
            for n in range(4):
                sl = slice(n * 512, (n + 1) * 512)
                for row, src in ((rowHe, expT_e), (rowHo, expT_o),
                                 (rowWe, expTW_e), (rowWo, expTW_o)):
                    ps = sps.tile([1, 512], dt.float32, tag="srow")
                    nc.tensor.matmul(ps[:], og[0:64], src[:, sl],
                                     start=True, stop=True)
                    nc.scalar.copy(row[:, sl], ps[:])

        # s_cpx2_{e,o}[w2*64+h] = sH + sW_{h%2}[(h//2)*64 + w]; in-place on sH
        for rowH, wpar in ((rowHe, 0), (rowHo, 1)):
            for rowW, hpar in ((rowWe, 0), (rowWo, 1)):
                ov = rowH[:].rearrange("p (w2 h) -> p w2 h", w2=32)[:, :, hpar::2]
                iv = rowW[:].rearrange("p (a ww) -> p a ww", a=32) \
                    .rearrange("p a (w2 r) -> p w2 r a", r=2)[:, :, wpar, :]
                nc.vector.tensor_add(ov, ov, iv)
        if cca_stop == "srows":
            for i, row in enumerate((rowHe, rowHo, rowWe, rowWo)):
                nc.sync.dma_start(dbg_out[0, i:i + 1, 0:2048], row[:])
            return
        # recip: rs_cpx2 into rowWe/rowWo
        nc.vector.reciprocal_approx_fast(out=rowWe[:], in_=rowHe[:])
        nc.vector.reciprocal_approx_fast(out=rowWo[:], in_=rowHo[:])
        # rs_rpx2[h2*64+w] = rs_cpx2_{w%2}[(w//2)*64 + 2*h2(+1)]; into rowHe/o
        for dst, hpar in ((rowHe, 0), (rowHo, 1)):
            for srcrow, wpar in ((rowWe, 0), (rowWo, 1)):
                ov = dst[:].rearrange("p (h2 w) -> p h2 w", h2=32)[:, :, wpar::2]
                iv = srcrow[:].rearrange("p (w2 hh) -> p w2 hh", w2=32) \
                    .rearrange("p w2 (h2 r) -> p h2 r w2", r=2)[:, :, hpar, :]
                nc.vector.tensor_copy(ov, iv)

        # broadcast + scale the exp tensors (all base-0 [64, 2048])
        for tgt, row in ((expT_e, rowWe), (expT_o, rowWo),
                         (expTW_e, rowHe), (expTW_o, rowHo)):
            scr = sp.tile([64, 2048], dt.float32, tag="scr", bufs=1)
            nc.gpsimd.partition_broadcast(scr[:], row[:])
            nc.vector.tensor_mul(tgt[:], tgt[:], scr[:])

        if cca_stop == "scale":
            with tc.tile_pool(name="dbge2", bufs=2) as dbge2:
                for i, t in enumerate((expT_e, expT_o, expTW_e, expTW_o)):
                    s1 = dbge2.tile([64, 2048], dt.float32)
                    nc.vector.tensor_copy(s1[:], t[:])
                    nc.sync.dma_start(dbg_out[i, 0:64, 0:2048], s1[:])
            return

        # ---- spatially-reordered unpadded feats copies ----
        pool_f = ctx.enter_context(tc.tile_pool(name="cca_f", bufs=1))
        frm = pool_f.tile([128, CT, HW], dt.float16)
        fcm = pool_f.tile([128, CT, HW], dt.float16)
        for ct in range(CT):
            nc.sync.dma_start(
                frm[:, ct, :].rearrange("p (h w) -> p h w", w=W),
                _pad_ap(fin, ct))
            eng = nc.vector if ct % 2 == 0 else nc.gpsimd
            eng.tensor_copy(
                fcm[:, ct, :].rearrange("p (w h) -> p w h", w=W),
                _pad_ap(fin, ct).rearrange("p h w -> p w h"))

        # ---- v projections + aggregation ----
        with (
            tc.tile_pool(name="cca_v", bufs=4) as vpool,
            tc.tile_pool(name="cca_vps", bufs=2, space="PSUM") as vps,
            tc.tile_pool(name="cca_aps", bufs=4, space="PSUM") as aps,
        ):
            for direction in range(2):  # 0: H (columns), 1: W (rows)
                fsrc = fcm if direction == 0 else frm
                exps = expT if direction == 0 else expTW
                for grp in range(8):
                    vt, vodd = [], []
                    for tt in range(4):
                        t = grp * 4 + tt
                        v = vpool.tile([128, C], dt.bfloat16, tag="vt")
                        ps = vps.tile([128, C], dt.float32, tag="vps")
                        for kt in range(CT):
                            nc.tensor.matmul(
                                ps[:], fsrc[:, kt, 128 * t:128 * (t + 1)],
                                vw[:, kt, :],
                                start=(kt == 0), stop=(kt == CT - 1))
                        nc.scalar.activation(v[:], ps[:], AF.Copy)
                        vo = vpool.tile([64, C], dt.bfloat16, tag="vodd")
                        nc.sync.dma_start(vo[:], v[64:128, :])
                        vt.append(v)
                        vodd.append(vo)
                    if cca_stop == "vproj":
                        continue
                    for cc in range(CT):
                        ps = aps.tile([128, 512], dt.float32, tag="agg")
                        for j in range(8):
                            x = grp * 8 + j  # column or row index
                            vsl = (vt[j // 2][0:64, cc * 128:(cc + 1) * 128]
                                   if x % 2 == 0 else
                                   vodd[j // 2][:, cc * 128:(cc + 1) * 128])
                            nc.tensor.matmul(
                                ps[:, j * 64:(j + 1) * 64], vsl,
                                exps[x % 2][:, (x // 2) * 64:(x // 2) * 64 + 64],
                                start=True, stop=True)
                        if direction == 0:
                            dview = fout[:, cc, :].rearrange(
                                "p (h w) -> p h w", w=WP)[
                                :, 1:1 + H, 1 + grp * 8:1 + grp * 8 + 8] \
                                .rearrange("p h w -> p w h")
                            fview = fin[:, cc, :].rearrange(
                                "p (h w) -> p h w", w=WP)[
                                :, 1:1 + H, 1 + grp * 8:1 + grp * 8 + 8] \
                                .rearrange("p h w -> p w h")
                            if cca_stop == "aggmm":
                                nc.vector.tensor_copy(
                                    dview,
                                    ps[:].rearrange("p (w h) -> p w h", w=8))
                            else:
                                nc.vector.scalar_tensor_tensor(
                                    out=dview,
                                    in0=ps[:].rearrange("p (w h) -> p w h", w=8),
                                    scalar=gvb[:, cc:cc + 1], in1=fview,
                                    op0=ALU.add, op1=ALU.add)
                        else:
                            dview = fout[:, cc, :].rearrange(
                                "p (h w) -> p h w", w=WP)[
                                :, 1 + grp * 8:1 + grp * 8 + 8, 1:1 + W]
                            nc.vector.tensor_add(
                                dview, dview,
                                ps[:].rearrange("p (h w) -> p h w", w=W))
                if direction == 0 and cca_stop in ("aggH", "vproj", "aggmm"):
                    return



# ---------------- host side ----------------

def _prep_inputs(x, conv1_w, bn1_g, bn1_b, bn1_m, bn1_v,
                 q_w, q_b, k_w, k_b, v_w, v_b, cca_gamma,
                 conv2_w, bn2_g, bn2_b, bn2_m, bn2_v):
    f16 = np.float16
    eps = 1e-5
    gamma = float(cca_gamma)
    x = np.asarray(x)
    conv1_w = np.asarray(conv1_w)
    conv2_w = np.asarray(conv2_w)
    assert np.max(np.abs(np.asarray(k_b))) == 0.0, "nonzero k bias unsupported"

    bn1_scale = (np.asarray(bn1_g) / np.sqrt(np.asarray(bn1_v) + eps)).astype(np.float32)
    bn1_shift = (np.asarray(bn1_b) - np.asarray(bn1_m) * bn1_scale).astype(np.float32)
    bn1_t = np.ascontiguousarray(
        np.stack([bn1_scale.reshape(CT, 128).T, bn1_shift.reshape(CT, 128).T],
                 axis=-1), np.float32)

    qk_t = np.concatenate([np.asarray(q_w).T, np.asarray(k_w).T], axis=1)
    qkw_t = np.ascontiguousarray(qk_t.reshape(CT, 128, 128), f16)
    qb_t = np.asarray(q_b).reshape(64, 1).astype(np.float32)
    vw_t = np.ascontiguousarray(np.asarray(v_w).T.reshape(CT, 128, C), f16)
    gvb_t = np.ascontiguousarray((gamma * np.asarray(v_b)).reshape(CT, 128).T,
                                 np.float32)
    og_t = np.full((64, 1), 1.0 / gamma, ml_dtypes.bfloat16)
    mask_t = np.ascontiguousarray((1.0 - np.eye(64)).astype(ml_dtypes.bfloat16))

    bn2_scale = (np.asarray(bn2_g) / np.sqrt(np.asarray(bn2_v) + eps)).astype(np.float32)
    bn2_shift = (np.asarray(bn2_b) - np.asarray(bn2_m) * bn2_scale).astype(np.float32)

    common = dict(qkw=qkw_t, qb=qb_t, vw=vw_t, gvb=gvb_t, og=og_t,
                  mask=mask_t, bn1=bn1_t)

    in_maps = []
    for core in range(N_CORES):
        b, half = core // 2, core % 2
        xs = x[b, half * 1024:(half + 1) * 1024].reshape(KT1, 128, HW).astype(f16)
        w1s = conv1_w[:, half * 1024:(half + 1) * 1024]
        w1s = w1s.reshape(C, KT1, 128, 3, 3).transpose(1, 3, 4, 2, 0) \
            .reshape(KT1, 9, 128, C).astype(f16)
        w2s = conv2_w[half * 256:(half + 1) * 256]
        w2s = w2s.reshape(256, CT, 128, 3, 3).transpose(1, 3, 4, 2, 0) \
            .reshape(CT, 9, 128, 256).astype(f16)
        bs = bn2_scale[half * 256:(half + 1) * 256].reshape(2, 128).T
        bh = bn2_shift[half * 256:(half + 1) * 256].reshape(2, 128).T
        bn2_t = np.ascontiguousarray(np.stack([bs, bh], axis=-1), np.float32)
        in_maps.append(dict(common, x=np.ascontiguousarray(xs),
                            w1=np.ascontiguousarray(w1s),
                            w2=np.ascontiguousarray(w2s), bn2=bn2_t))
    return in_maps


def _get_compiled(debug_stage, cca_stop=None):
    key = (debug_stage, cca_stop)
    if key not in _COMPILED:
        _COMPILED[key] = build_kernel(debug_stage, cca_stop)
    return _COMPILED[key]


def run(inputs, debug_stage=None, trace=False, cca_stop=None):
    nc = _get_compiled(debug_stage, cca_stop)
    in_maps = _prep_inputs(**inputs)
    return run_bass_kernel_spmd(nc, in_maps, list(range(N_CORES)), trace=trace)


def kernel(**inputs):
    res = run(inputs, debug_stage=DEBUG_STAGE)
    out = np.empty((B, C, H, W), np.float32)
    if DEBUG_STAGE is None:
        for core in range(N_CORES):
            b, half = core // 2, core % 2
            out[b, half * 256:(half + 1) * 256] = \
                res.results[core]["out"].reshape(256, H, W)
    else:
        for b in range(B):
            out[b] = res.results[2 * b]["out"].reshape(C, H, W)
    return out



# revision 27
# speedup vs baseline: 1.3691x; 1.3691x over previous
"""CCNet unit (conv3x3 -> BN/ReLU -> 2x criss-cross attention -> conv3x3 ->
BN/ReLU) on 8 Trainium2 NeuronCores.

Sharding (SPMD-symmetric program; per-core differences live only in data):
  core = 2*b + half   (b = sample 0..3, half = 0/1)
  - conv1 (Cin=2048): input-channel split across the pair; partial sums
    pair-AllReduce'd (fp16), then BN+ReLU on both cores.
  - CCA x2: computed redundantly by both pair members (cheap vs conv1).
  - conv2 (Cout=512): output-channel split via sharded weights.

All matmuls fp16 (full PE rate), PSUM accumulates fp32.

Attention layouts (parity-packed across the 128 partitions):
  q2/k2 [128, HW] : q/k replicated across partition halves (for PE
    row-group packing via tile_position)
  expT  [128, 2048] : exp(eH^T)[i, (w//2)*64+h]; even w at partitions
    0:64, odd w at 64:128 (diag-masked)
  expTW [128, 2048] : exp(eW^T)[j, (h//2)*64+w]; even h top, odd h bottom
  v tiles [128, C]  : pixel-parity packed by construction of the v-proj
Softmax has no max-subtraction (logits bounded ~ +-5); normalization
(gamma/s) is folded into expT/expTW before aggregation.

Concurrent small matmuls are packed into PE quadrants with explicit
tile_position; each concurrent matmul gets its own PSUM tile (same-tile
cross-column-group packing crashes the PE).
"""

import sys

sys.path.insert(0, "/opt/trn_rl_repo")

import numpy as np
import ml_dtypes

import concourse.bacc as bacc
import concourse.mybir as mybir
import concourse.tile as tile
from concourse.bass_utils import run_bass_kernel_spmd

dt = mybir.dt
AF = mybir.ActivationFunctionType
ALU = mybir.AluOpType

B, CIN, C, CR, H, W = 4, 2048, 512, 64, 64, 64
HW = H * W
WP = W + 2
PADPX = (H + 2) * WP
INT0 = WP + 1
KT1 = CIN // 2 // 128   # 8
CT = C // 128           # 4
N_CORES = 8
PAIRS = [[0, 1], [2, 3], [4, 5], [6, 7]]

_COMPILED = {}


def _pad_ap(t, ct):
    """[128, 64, 64] interior view of padded feats tile channel-tile ct."""
    return t[:, ct, INT0:INT0 + WP * H].rearrange("p (h w) -> p h w", w=WP)[:, :, 0:W]


def _chunk_rhs(t, ct, h0, off):
    """[128, 8, 64] rhs AP: 8 output rows from h0, reading offset `off`."""
    r0, c0 = h0 + off // WP, off % WP
    v = t[:, ct, :].rearrange("p (h w) -> p h w", w=WP)
    return v[:, r0:r0 + 8, c0:c0 + 64]


def build_kernel(debug_stage=None):
    nc = bacc.Bacc("TRN2", target_bir_lowering=False, debug=False,
                   num_devices=N_CORES)

    x_in = nc.dram_tensor("x", [KT1, 128, HW], dt.float16, kind="ExternalInput").ap()
    w1_in = nc.dram_tensor("w1", [KT1, 9, 128, C], dt.float16, kind="ExternalInput").ap()
    bn1_in = nc.dram_tensor("bn1", [128, CT, 2], dt.float32, kind="ExternalInput").ap()
    qkw_in = nc.dram_tensor("qkw", [CT, 128, 128], dt.float16, kind="ExternalInput").ap()
    qb_in = nc.dram_tensor("qb", [64, 1], dt.float32, kind="ExternalInput").ap()
    vw_in = nc.dram_tensor("vw", [CT, 128, C], dt.float16, kind="ExternalInput").ap()
    gvb_in = nc.dram_tensor("gvb", [128, CT], dt.float32, kind="ExternalInput").ap()
    og_in = nc.dram_tensor("og", [64, 1], dt.bfloat16, kind="ExternalInput").ap()
    mask_in = nc.dram_tensor("mask", [64, 64], dt.bfloat16, kind="ExternalInput").ap()
    w2_in = nc.dram_tensor("w2", [CT, 9, 128, 256], dt.float16, kind="ExternalInput").ap()
    bn2_in = nc.dram_tensor("bn2", [128, 2, 2], dt.float32, kind="ExternalInput").ap()
    if debug_stage is None:
        out_t = nc.dram_tensor("out", [2, 128, HW], dt.float32, kind="ExternalOutput").ap()
    else:
        out_t = nc.dram_tensor("out", [CT, 128, HW], dt.float32, kind="ExternalOutput").ap()

    with tile.TileContext(nc) as tc:
        _emit(nc, tc, x_in, w1_in, bn1_in, qkw_in, qb_in, vw_in,
              gvb_in, og_in, mask_in, w2_in, bn2_in, out_t, debug_stage)
    nc.compile()
    return nc


def _emit_debug_feats(nc, tc, feats, out_t):
    with tc.tile_pool(name="dbg", bufs=4) as dbg:
        for ct in range(CT):
            st = dbg.tile([128, HW], dt.float32)
            nc.vector.tensor_copy(
                st[:].rearrange("p (h w) -> p h w", w=W), _pad_ap(feats, ct))
            nc.sync.dma_start(out_t[ct], st[:])


def _emit(nc, tc, x_in, w1_in, bn1_in, qkw_in, qb_in, vw_in,
          gvb_in, og_in, mask_in, w2_in, bn2_in, out_t, debug_stage=None):
    from contextlib import ExitStack

    ctx = ExitStack()
    with ctx:
        pool_feats = ctx.enter_context(tc.tile_pool(name="feats", bufs=1))
        featsA = pool_feats.tile([128, CT, PADPX], dt.float16)

        pool_const = ctx.enter_context(tc.tile_pool(name="const", bufs=1))
        bn1 = pool_const.tile([128, CT, 2], dt.float32)
        qkw = pool_const.tile([128, CT, 128], dt.float16)
        qb = pool_const.tile([64, 1], dt.float32)
        vw = pool_const.tile([128, CT, C], dt.float16)
        gvb = pool_const.tile([128, CT], dt.float32)
        og2 = pool_const.tile([128, 1], dt.bfloat16)
        mask = pool_const.tile([128, 64], dt.bfloat16)

        # ---------------- conv1 ----------------
        with (
            tc.tile_pool(name="c1", bufs=1) as c1,
            tc.tile_pool(name="c1ps", bufs=8, space="PSUM") as c1ps,
            tc.tile_pool(name="c1st", bufs=4) as c1st,
            tc.tile_pool(name="c1dram", bufs=1, space="DRAM") as c1dram,
        ):
            x_pad = c1.tile([128, KT1, PADPX], dt.float16)
            w1 = c1.tile([128, KT1, 9, C], dt.float16)
            # per-kt memset + DMA so kt0 is ready in ~5us
            for kt in range(KT1):
                nc.vector.memset(x_pad[:, kt, :], 0.0)
                nc.sync.dma_start(w1[:, kt, :, :],
                                  w1_in[kt].rearrange("t p c -> p t c"))
                nc.sync.dma_start(_pad_ap(x_pad, kt),
                                  x_in[kt].rearrange("p (h w) -> p h w", w=W))

            # consts (needed later; issue after kt0 traffic)
            nc.sync.dma_start(bn1[:], bn1_in[:])
            nc.sync.dma_start(qkw[:], qkw_in[:].rearrange("k p c -> p k c"))
            nc.sync.dma_start(qb[:], qb_in[:])
            nc.sync.dma_start(vw[:], vw_in[:].rearrange("k p c -> p k c"))
            nc.sync.dma_start(gvb[:], gvb_in[:])
            nc.sync.dma_start(og2[0:64], og_in[:])
            nc.sync.dma_start(og2[64:128], og_in[:])
            nc.sync.dma_start(mask[0:64, :], mask_in[:])
            nc.sync.dma_start(mask[64:128, :], mask_in[:])
            nc.vector.memset(featsA[:], 0.0)

            HW2 = HW // 2
            partial = c1dram.tile([CT, 2, 128, HW2], dt.float16)
            reduced = c1dram.tile([CT, 2, 128, HW2], dt.float16)

            for mt in range(CT):
                pss = []
                for h0 in range(0, H, 8):
                    ps = c1ps.tile([128, 512], dt.float32, tag=f"c1ps{h0}",
                                   bufs=1)
                    pss.append(ps)
                for kt in range(KT1):
                    for i, h0 in enumerate(range(0, H, 8)):
                        for tap in range(9):
                            dy, dx = tap // 3, tap % 3
                            nc.tensor.matmul(
                                pss[i][:],
                                w1[:, kt, tap, mt * 128:(mt + 1) * 128],
                                _chunk_rhs(x_pad, kt, h0, dy * WP + dx),
                                start=(kt == 0 and tap == 0),
                                stop=(kt == KT1 - 1 and tap == 8))
                for i, h0 in enumerate(range(0, H, 8)):
                    st = c1st.tile([128, 512], dt.float16)
                    nc.scalar.activation(st[:], pss[i][:], AF.Copy)
                    nc.sync.dma_start(
                        partial[mt, h0 // 32, :, (h0 % 32) * W:(h0 % 32 + 8) * W],
                        st[:])

                # collective: whole for mt<3, split in 2 for the last mt
                if mt < CT - 1:
                    nc.gpsimd.collective_compute(
                        "AllReduce", ALU.add, replica_groups=PAIRS,
                        ins=[partial[mt]], outs=[reduced[mt]])
                else:
                    for c in range(2):
                        nc.gpsimd.collective_compute(
                            "AllReduce", ALU.add, replica_groups=PAIRS,
                            ins=[partial[mt, c]], outs=[reduced[mt, c]])
                for h0 in range(0, H, 8):
                    red_sb = c1st.tile([128, 512], dt.float16, tag="redsb")
                    nc.sync.dma_start(
                        red_sb[:],
                        reduced[mt, h0 // 32, :, (h0 % 32) * W:(h0 % 32 + 8) * W])
                    nc.scalar.activation(
                        _chunk_rhs(featsA, mt, h0, INT0),
                        red_sb[:].rearrange("p (h w) -> p h w", w=W),
                        AF.Relu, bias=bn1[:, mt, 1:2], scale=bn1[:, mt, 0:1])

        # ---------------- CCA x2 ----------------
        if debug_stage == "feats1":
            _emit_debug_feats(nc, tc, featsA, out_t)
            return
        pool_fb = ctx.enter_context(tc.tile_pool(name="featsB", bufs=1))
        featsB = pool_fb.tile([128, CT, PADPX], dt.float16)
        nc.vector.memset(featsB[:], 0.0)
        _emit_cca(nc, tc, featsA, featsB, qkw, qb, vw, gvb, og2, mask,
                  debug_stage=debug_stage, dbg_out=out_t)
        if debug_stage in ("qk", "exp", "scaled", "rows"):
            return
        if debug_stage == "cca1":
            _emit_debug_feats(nc, tc, featsB, out_t)
            return
        _emit_cca(nc, tc, featsB, featsA, qkw, qb, vw, gvb, og2, mask)
        if debug_stage == "cca2":
            _emit_debug_feats(nc, tc, featsA, out_t)
            return

        # ---------------- conv2 ----------------
        with (
            tc.tile_pool(name="c2", bufs=1) as c2,
            tc.tile_pool(name="c2ps", bufs=4, space="PSUM") as c2ps,
            tc.tile_pool(name="c2st", bufs=4) as c2st,
        ):
            w2 = c2.tile([128, CT, 9, 256], dt.float16)
            bn2 = c2.tile([128, 2, 2], dt.float32)
            nc.sync.dma_start(bn2[:], bn2_in[:])
            for kt in range(CT):
                nc.sync.dma_start(w2[:, kt, :, :],
                                  w2_in[kt].rearrange("t p c -> p t c"))
            for h0 in range(0, H, 8):
                for mt in range(2):
                    ps = c2ps.tile([128, 512], dt.float32)
                    i = 0
                    for kt in range(CT):
                        for tap in range(9):
                            dy, dx = tap // 3, tap % 3
                            nc.tensor.matmul(
                                ps[:],
                                w2[:, kt, tap, mt * 128:(mt + 1) * 128],
                                _chunk_rhs(featsA, kt, h0, dy * WP + dx),
                                start=(i == 0), stop=(i == CT * 9 - 1))
                            i += 1
                    st = c2st.tile([128, 512], dt.float32)
                    nc.scalar.activation(st[:], ps[:], AF.Relu,
                                         bias=bn2[:, mt, 1:2],
                                         scale=bn2[:, mt, 0:1])
                    nc.sync.dma_start(out_t[mt, :, h0 * W:(h0 + 8) * W], st[:])


def _emit_cca(nc, tc, fin, fout, qkw, qb, vw, gvb, og2, mask,
              debug_stage=None, dbg_out=None):
    """fout = gamma*cca(fin) + fin (interior; fout border must be zero)."""
    from contextlib import ExitStack

    def _dump(tiles, n_free):
        with tc.tile_pool(name="dbgc", bufs=1) as dbgp:
            ck = 1024
            for i, t in enumerate(tiles):
                for c0 in range(0, n_free, ck):
                    n = min(ck, n_free - c0)
                    st = dbgp.tile([128, 1024], dt.float32, tag="dbgt",
                                   bufs=1)
                    nc.vector.tensor_copy(st[:, 0:n], t[:, c0:c0 + n])
                    nc.sync.dma_start(dbg_out[i, :, c0:c0 + n], st[:, 0:n])

    ctx = ExitStack()
    with ctx:
        sb = ctx.enter_context(tc.tile_pool(name="cca_sb", bufs=1))
        sp = ctx.enter_context(tc.tile_pool(name="cca_small", bufs=2))

        expT = sb.tile([128, 2048], dt.bfloat16)
        expTW = sb.tile([128, 2048], dt.bfloat16)
        q2 = sb.tile([128, HW], dt.float16)
        k2 = sb.tile([128, HW], dt.float16)
        fcm = sb.tile([128, CT, HW], dt.float16)
        frm = sb.tile([128, CT, HW], dt.float16)

        # ---- reordered feats copies (v-proj lhsT must be contiguous);
        # overlap the qk projections ----
        for ct in range(CT):
            eng = nc.gpsimd if ct % 2 == 0 else nc.vector
            eng.tensor_copy(
                fcm[:, ct, :].rearrange("p (w h) -> p w h", w=W),
                _pad_ap(fin, ct).rearrange("p h w -> p w h"))
            nc.sync.dma_start(
                frm[:, ct, :].rearrange("p (h w) -> p h w", w=W),
                _pad_ap(fin, ct))

        with (
            tc.tile_pool(name="cca_qkps", bufs=4, space="PSUM") as qkps,
        ):
            # ---- q, k projections; q lands 0:64, k lands 64:128 ----
            for n in range(8):
                ps = qkps.tile([128, 512], dt.float32)
                for kt in range(CT):
                    nc.tensor.matmul(
                        ps[:], qkw[:, kt, :], _chunk_rhs(fin, kt, n * 8, INT0),
                        start=(kt == 0), stop=(kt == CT - 1))
                sl = slice(n * 512, (n + 1) * 512)
                nc.scalar.activation(q2[0:64, sl], ps[0:64, :],
                                     AF.Identity, bias=qb[:], scale=1.0)
                nc.scalar.activation(k2[64:128, sl], ps[64:128, :], AF.Copy)
                # replicate to the other partition half (for row-group packing)
                nc.sync.dma_start(q2[64:128, sl], q2[0:64, sl])
                nc.sync.dma_start(k2[0:64, sl], k2[64:128, sl])

        if debug_stage == "qk":
            _dump([q2[:], k2[:]], HW)
            return

        # ---- e^T matmuls + exp: 4 PE quadrants concurrently ----
        # residue r = w%4 -> quadrant; each r gets its own PSUM tile.
        # r0:(0,0) lo ops, out 0:64   r1:(0,64) lo ops, out 64:128
        # r2:(64,0) hi ops, out 0:64  r3:(64,64) hi ops, out 64:128
        with tc.tile_pool(name="cca_eps", bufs=2, space="PSUM") as eps:
            for direction in range(2):
                tgt = expT if direction == 0 else expTW
                if direction == 0:
                    kv = k2[:].rearrange("p (h w) -> p w h", w=W)
                    qv = q2[:].rearrange("p (h w) -> p w h", w=W)
                else:
                    kv = k2[:].rearrange("p (h w) -> p h w", w=W)
                    qv = q2[:].rearrange("p (h w) -> p h w", w=W)
                for sbk in range(2):  # superblock of 32 columns/rows
                    # full-bank [128, 512] tiles: concurrent quadrant MMs
                    # must not share a PSUM bank
                    pr = []
                    for r in range(4):
                        t = eps.tile([128, 512], dt.float32, tag=f"eps{r}")
                        pr.append(t)
                    for t8 in range(8):
                        for r in range(4):
                            w = sbk * 32 + 4 * t8 + r
                            lo = r < 2
                            pslc = slice(0, 64) if r % 2 == 0 else slice(64, 128)
                            opslc = slice(0, 64) if lo else slice(64, 128)
                            pos = (0 if lo else 64, 0 if r % 2 == 0 else 64)
                            nc.tensor.matmul(
                                pr[r][pslc, t8 * 64:(t8 + 1) * 64],
                                kv[opslc, w, :], qv[opslc, w, :],
                                start=True, stop=True, tile_position=pos)
                    # exp drains: tile r holds w = sbk*32 + 4*t8 + r
                    # target col = (w//2)*64 = sbk*1024 + t8*128 + (r//2)*64
                    base = sbk * 1024
                    for r in range(4):
                        pslc = slice(0, 64) if r % 2 == 0 else slice(64, 128)
                        c0 = (r // 2) * 64
                        ov = tgt[pslc, base:base + 1024] \
                            .rearrange("p (t b) -> p t b", b=128)[:, :, c0:c0 + 64]
                        nc.scalar.activation(
                            ov, pr[r][pslc, :].rearrange("p (t b) -> p t b", b=64),
                            AF.Exp)
            # diag mask on expT (criss-cross self position)
            nc.vector.tensor_mul(
                expT[:].rearrange("p (a b) -> p a b", b=64),
                expT[:].rearrange("p (a b) -> p a b", b=64),
                mask[:, None, :].broadcast_to((128, 32, 64)))

        if debug_stage == "exp":
            _dump([expT[:], expTW[:]], 2048)
            return

        # ---- denominators (rows [1, 2048]) ----
        rowHe = sp.tile([1, 2048], dt.float32, tag="rowHe", bufs=1)
        rowHo = sp.tile([1, 2048], dt.float32, tag="rowHo", bufs=1)
        rowWe = sp.tile([1, 2048], dt.float32, tag="rowWe", bufs=1)
        rowWo = sp.tile([1, 2048], dt.float32, tag="rowWo", bufs=1)
        with tc.tile_pool(name="cca_sps", bufs=2, space="PSUM") as sps:
            for n in range(4):
                sl = slice(n * 512, (n + 1) * 512)
                for row, src, hi in ((rowHe, expT, False), (rowHo, expT, True),
                                     (rowWe, expTW, False), (rowWo, expTW, True)):
                    opslc = slice(64, 128) if hi else slice(0, 64)
                    pos = (64 if hi else 0, 0)
                    ps = sps.tile([1, 512], dt.float32,
                                  tag=f"srow{'h' if hi else 'l'}")
                    nc.tensor.matmul(ps[:], og2[opslc], src[opslc, sl],
                                     start=True, stop=True, tile_position=pos)
                    nc.scalar.copy(row[:, sl], ps[:])

        # s_cpx2_{e,o}[w2*64+h] = sH + sW_{h%2}[(h//2)*64 + w]; in-place on sH
        for rowH, wpar in ((rowHe, 0), (rowHo, 1)):
            for rowW, hpar in ((rowWe, 0), (rowWo, 1)):
                ov = rowH[:].rearrange("p (w2 h) -> p w2 h", w2=32)[:, :, hpar::2]
                iv = rowW[:].rearrange("p (a ww) -> p a ww", a=32) \
                    .rearrange("p a (w2 r) -> p w2 r a", r=2)[:, :, wpar, :]
                nc.vector.tensor_add(ov, ov, iv)
        # recip: rs_cpx2 into rowWe/rowWo
        nc.vector.reciprocal_approx_fast(out=rowWe[:], in_=rowHe[:])
        nc.vector.reciprocal_approx_fast(out=rowWo[:], in_=rowHo[:])
        # rs_rpx2[h2*64+w] = rs_cpx2_{w%2}[(w//2)*64 + 2*h2(+1)]; into rowHe/o
        for dst, hpar in ((rowHe, 0), (rowHo, 1)):
            for srcrow, wpar in ((rowWe, 0), (rowWo, 1)):
                ov = dst[:].rearrange("p (h2 w) -> p h2 w", h2=32)[:, :, wpar::2]
                iv = srcrow[:].rearrange("p (w2 hh) -> p w2 hh", w2=32) \
                    .rearrange("p w2 (h2 r) -> p h2 r w2", r=2)[:, :, hpar, :]
                nc.vector.tensor_copy(ov, iv)

        if debug_stage == "rows":
            for i, row in enumerate((rowHe, rowHo, rowWe, rowWo)):
                nc.sync.dma_start(dbg_out[0, i:i + 1, 0:2048], row[:])
            return

        # broadcast + scale the exp tensors: top half (even) and bottom (odd).
        # partition_broadcast always fills from partition 0, so broadcast the
        # full 128 partitions and multiply the matching half each time.
        for tgt, rowe, rowo in ((expT, rowWe, rowWo), (expTW, rowHe, rowHo)):
            scr = sp.tile([128, 2048], dt.float32, tag="scr", bufs=1)
            nc.gpsimd.partition_broadcast(scr[:], rowe[:])
            nc.vector.tensor_mul(tgt[0:64, :], tgt[0:64, :], scr[0:64, :])
            scr2 = sp.tile([128, 2048], dt.float32, tag="scr", bufs=1)
            nc.gpsimd.partition_broadcast(scr2[:], rowo[:])
            nc.vector.tensor_mul(tgt[64:128, :], tgt[64:128, :], scr2[64:128, :])

        if debug_stage == "scaled":
            _dump([expT[:], expTW[:]], 2048)
            return

        # ---- v projections + aggregation ----
        with (
            tc.tile_pool(name="cca_v", bufs=6) as vpool,
            tc.tile_pool(name="cca_vps", bufs=2, space="PSUM") as vps,
            tc.tile_pool(name="cca_aps", bufs=2, space="PSUM") as aps,
        ):
            for direction in range(2):  # 0: H (columns), 1: W (rows)
                exps = expT if direction == 0 else expTW
                for grp in range(8):
                    vt = []
                    for tt in range(4):
                        t = grp * 4 + tt
                        v = vpool.tile([128, C], dt.bfloat16, tag="vt")
                        ps = vps.tile([128, C], dt.float32, tag="vps")
                        for kt in range(CT):
                            if direction == 0:
                                lhsT = fcm[:, kt, 128 * t:128 * (t + 1)]
                            else:
                                lhsT = frm[:, kt, 128 * t:128 * (t + 1)]
                            nc.tensor.matmul(
                                ps[:], lhsT, vw[:, kt, :],
                                start=(kt == 0), stop=(kt == CT - 1))
                        nc.scalar.activation(v[:], ps[:], AF.Copy)
                        vt.append(v)
                    for ccp in range(2):
                        # full-bank [128, 512] tiles, each holding the agg
                        # outputs of a cc-pair; A: even cols/rows, B: odd
                        psA = aps.tile([128, 512], dt.float32, tag="aggA")
                        psB = aps.tile([128, 512], dt.float32, tag="aggB")
                        for j in range(8):
                            for cci in range(2):
                                cc = 2 * ccp + cci
                                x = grp * 8 + j  # column or row index
                                hi = (x % 2 == 1)
                                opslc = slice(64, 128) if hi else slice(0, 64)
                                pst = psB if hi else psA
                                o0 = cci * 256 + (j // 2) * 64
                                nc.tensor.matmul(
                                    pst[:, o0:o0 + 64],
                                    vt[j // 2][opslc, cc * 128:(cc + 1) * 128],
                                    exps[opslc, (x // 2) * 64:(x // 2) * 64 + 64],
                                    start=True, stop=True,
                                    tile_position=(64 if hi else 0, 0))
                        for cci in range(2):
                            cc = 2 * ccp + cci
                            csl = slice(cci * 256, cci * 256 + 256)
                            if direction == 0:
                                base = fout[:, cc, :].rearrange(
                                    "p (h w) -> p h w", w=WP)[
                                    :, 1:1 + H, 1 + grp * 8:1 + grp * 8 + 8] \
                                    .rearrange("p h w -> p w h")
                                fbase = fin[:, cc, :].rearrange(
                                    "p (h w) -> p h w", w=WP)[
                                    :, 1:1 + H, 1 + grp * 8:1 + grp * 8 + 8] \
                                    .rearrange("p h w -> p w h")
                                for par, pst in ((0, psA), (1, psB)):
                                    nc.vector.scalar_tensor_tensor(
                                        out=base[:, par::2, :],
                                        in0=pst[:, csl].rearrange(
                                            "p (w h) -> p w h", w=4),
                                        scalar=gvb[:, cc:cc + 1],
                                        in1=fbase[:, par::2, :],
                                        op0=ALU.add, op1=ALU.add)
                            else:
                                base = fout[:, cc, :].rearrange(
                                    "p (h w) -> p h w", w=WP)[
                                    :, 1 + grp * 8:1 + grp * 8 + 8, 1:1 + W]
                                for par, pst in ((0, psA), (1, psB)):
                                    nc.vector.tensor_add(
                                        base[:, par::2, :], base[:, par::2, :],
                                        pst[:, csl].rearrange(
                                            "p (h w) -> p h w", w=W))


# ---------------- host side ----------------

def _prep_inputs(x, conv1_w, bn1_g, bn1_b, bn1_m, bn1_v,
                 q_w, q_b, k_w, k_b, v_w, v_b, cca_gamma,
                 conv2_w, bn2_g, bn2_b, bn2_m, bn2_v):
    f16 = np.float16
    eps = 1e-5
    gamma = float(cca_gamma)
    x = np.asarray(x)
    conv1_w = np.asarray(conv1_w)
    conv2_w = np.asarray(conv2_w)
    assert np.max(np.abs(np.asarray(k_b))) == 0.0, "nonzero k bias unsupported"

    bn1_scale = (np.asarray(bn1_g) / np.sqrt(np.asarray(bn1_v) + eps)).astype(np.float32)
    bn1_shift = (np.asarray(bn1_b) - np.asarray(bn1_m) * bn1_scale).astype(np.float32)
    bn1_t = np.ascontiguousarray(
        np.stack([bn1_scale.reshape(CT, 128).T, bn1_shift.reshape(CT, 128).T],
                 axis=-1), np.float32)

    qk_t = np.concatenate([np.asarray(q_w).T, np.asarray(k_w).T], axis=1)
    qkw_t = np.ascontiguousarray(qk_t.reshape(CT, 128, 128), f16)
    qb_t = np.asarray(q_b).reshape(64, 1).astype(np.float32)
    vw_t = np.ascontiguousarray(np.asarray(v_w).T.reshape(CT, 128, C), f16)
    gvb_t = np.ascontiguousarray((gamma * np.asarray(v_b)).reshape(CT, 128).T,
                                 np.float32)
    og_t = np.full((64, 1), 1.0 / gamma, ml_dtypes.bfloat16)
    mask_t = np.ascontiguousarray((1.0 - np.eye(64)).astype(ml_dtypes.bfloat16))

    bn2_scale = (np.asarray(bn2_g) / np.sqrt(np.asarray(bn2_v) + eps)).astype(np.float32)
    bn2_shift = (np.asarray(bn2_b) - np.asarray(bn2_m) * bn2_scale).astype(np.float32)

    common = dict(qkw=qkw_t, qb=qb_t, vw=vw_t, gvb=gvb_t, og=og_t,
                  mask=mask_t, bn1=bn1_t)

    in_maps = []
    for core in range(N_CORES):
        b, half = core // 2, core % 2
        xs = x[b, half * 1024:(half + 1) * 1024].reshape(KT1, 128, HW).astype(f16)
        w1s = conv1_w[:, half * 1024:(half + 1) * 1024]
        w1s = w1s.reshape(C, KT1, 128, 3, 3).transpose(1, 3, 4, 2, 0) \
            .reshape(KT1, 9, 128, C).astype(f16)
        w2s = conv2_w[half * 256:(half + 1) * 256]
        w2s = w2s.reshape(256, CT, 128, 3, 3).transpose(1, 3, 4, 2, 0) \
            .reshape(CT, 9, 128, 256).astype(f16)
        bs = bn2_scale[half * 256:(half + 1) * 256].reshape(2, 128).T
        bh = bn2_shift[half * 256:(half + 1) * 256].reshape(2, 128).T
        bn2_t = np.ascontiguousarray(np.stack([bs, bh], axis=-1), np.float32)
        in_maps.append(dict(common, x=np.ascontiguousarray(xs),
                            w1=np.ascontiguousarray(w1s),
                            w2=np.ascontiguousarray(w2s), bn2=bn2_t))
    return in_maps


def _get_compiled(debug_stage=None):
    if debug_stage not in _COMPILED:
        _COMPILED[debug_stage] = build_kernel(debug_stage)
    return _COMPILED[debug_stage]


def run(inputs, trace=False, debug_stage=None, **kw):
    nc = _get_compiled(debug_stage)
    in_maps = _prep_inputs(**inputs)
    return run_bass_kernel_spmd(nc, in_maps, list(range(N_CORES)), trace=trace)


DEBUG_STAGE = None


def kernel(**inputs):
    res = run(inputs)
    out = np.empty((B, C, H, W), np.float32)
    for core in range(N_CORES):
        b, half = core // 2, core % 2
        out[b, half * 256:(half + 1) * 256] = \
            res.results[core]["out"].reshape(256, H, W)
    return out
